# revision 31
# baseline (speedup 1.0000x reference)
"""GTN-Rec on 8 TRN2 NeuronCores — v2.

2D sharding: 4 N-shards (500 columns each) x 2 batch-halves (32 batches).
core c: g = c % 4 (N-shard), h = c // 4 (batch half). Collectives run
within the 4-core groups [0-3] and [4-7].

Chain y3 = ((x@a0)@b0)@a20 computed transposed, fp8e4m3 DoubleRow
matmuls (k=250 per instruction). Mixtures a0/b0/a20 are built on-device
from the core's A column shard (2 scalar_tensor_tensor ops per mix,
largest softmax weight folded into the psum-evac scale). Activations
AllGather in fp8 within the 4-group between stages. Value-range folds:
y2 scaled by 1/128 (fp8), enc by 1/4096 (fp16, lin_w pre-scaled 4096x).

Basket partial ReduceScatters (fp32) to 8 batches per core. LSTM runs
with gates resident in PSUM (Wih@basket + bias precomputed there; the
per-step Whh matmuls accumulate in place), tanh done as 2*sigmoid(2x)-1
so all four gates take one Sigmoid activation, c tracked as c'=2c.
Scoring sigmoid(last @ Wscore^T) * ((1-a) + a*scale) per core; host
concatenates the [8, 2000] outputs in core order.
"""
import os

import numpy as np

N, E, B, S, D, U = 2000, 3, 64, 30, 128, 128
P = 8
NG = 4               # N-shard groups
NC = N // NG         # 500 columns per shard
KP = 125             # partition tile
JT = 8               # DoubleRow k-pair tiles (2000 = 8 * 2 * 125)
BH = B // 2          # 32 batches per half
BSH = BH * S         # 960 columns per half
BL = B // P          # 8 batches per core
GW = BL // 2         # 4 per LSTM group
ALPHA = 0.5
S_Y2 = 128.0         # y2 fp8 range scale
S_ENC = 4096.0       # enc/basket fp16 range scale

_CACHE = {}


def _softmax_row0(w):
    w = np.asarray(w, np.float64)
    m = w - w.max(axis=1, keepdims=True)
    e = np.exp(m)
    return (e / e.sum(axis=1, keepdims=True))[0]


def _mix_fold(w):
    """Order planes ascending by weight; returns (idx, r0, r1, fold) with
    mix = fold * ((A[i0]*r0 + A[i1])*r1 + A[i2]), all ratios <= 1."""
    idx = list(np.argsort(w))
    i0, i1, i2 = idx
    r0 = float(w[i0] / w[i1])
    r1 = float(w[i1] / w[i2])
    return (i0, i1, i2), r0, r1, float(w[i2])


def _build(mixes, dbg=False):
    import concourse.bacc as bacc
    import concourse.tile as tile
    import concourse.mybir as mybir
    from concourse.alu_op_type import AluOpType
    from contextlib import ExitStack

    f32, f16, f8 = mybir.dt.float32, mybir.dt.float16, mybir.dt.float8e4
    AF = mybir.ActivationFunctionType
    DR = mybir.MatmulPerfMode.DoubleRow

    (p1, r10, r11, fold1) = _mix_fold(mixes[0])
    (p2, r20, r21, fold2) = _mix_fold(mixes[1])
    (p3, r30, r31, fold3) = _mix_fold(mixes[2])
    s1 = fold1                    # stage-1 evac scale
    s2 = fold2 / S_Y2             # stage-2 evac scale
    s3 = S_Y2 * fold3 / S_ENC     # stage-3 relu scale

    nc = bacc.Bacc("TRN2", target_bir_lowering=False, debug=False, num_devices=P)

    ap_in = nc.dram_tensor("ap", [E, KP, JT * 2 * NC], f16, kind="ExternalInput").ap()
    xt_in = nc.dram_tensor("xt", [KP, JT, 2, 2, 480], f8, kind="ExternalInput").ap()
    xto_in = nc.dram_tensor("xto", [KP, 4, 2, 480], f16, kind="ExternalInput").ap()
    linw_in = nc.dram_tensor("linw", [KP, 4, D], f16, kind="ExternalInput").ap()
    lbt_in = nc.dram_tensor("lbt", [D, 1], f32, kind="ExternalInput").ap()
    ntt_in = nc.dram_tensor("ntt", [KP, 1], f32, kind="ExternalInput").ap()
    wih_in = nc.dram_tensor("wih", [D, 4 * U], f16, kind="ExternalInput").ap()
    whh_in = nc.dram_tensor("whh", [U, 4 * U], f16, kind="ExternalInput").ap()
    gb_in = nc.dram_tensor("gb", [1, 4 * U], f32, kind="ExternalInput").ap()
    wsc_in = nc.dram_tensor("wsc", [U, N], f16, kind="ExternalInput").ap()
    mp_in = nc.dram_tensor("mp", [U, S * BL], mybir.dt.uint8, kind="ExternalInput").ap()
    h0_in = nc.dram_tensor("h0t", [U, BL], f16, kind="ExternalInput").ap()
    c0_in = nc.dram_tensor("c0t", [U, BL], f32, kind="ExternalInput").ap()
    wb_in = nc.dram_tensor("wb", [BL, N], f32, kind="ExternalInput").ap()
    out_t = nc.dram_tensor("out", [BL, N], f32, kind="ExternalOutput").ap()
    if dbg:
        d_enc = nc.dram_tensor("d_enc", [KP, 4, 2, 480], f16, kind="ExternalOutput").ap()
        d_bkc = nc.dram_tensor("d_bkc", [D, S * BL], f16, kind="ExternalOutput").ap()
        d_last = nc.dram_tensor("d_last", [U, BL], f16, kind="ExternalOutput").ap()

    G1 = [[0, 1, 2, 3], [4, 5, 6, 7]]

    with ExitStack() as ctx:
        tc = ctx.enter_context(tile.TileContext(nc))
        wp = ctx.enter_context(tc.tile_pool(name="w", bufs=1))
        xp = ctx.enter_context(tc.tile_pool(name="x", bufs=1))
        ep = ctx.enter_context(tc.tile_pool(name="e", bufs=1))
        pp = ctx.enter_context(tc.tile_pool(name="ps", bufs=4, space="PSUM"))
        dr = ctx.enter_context(tc.tile_pool(name="dr", bufs=1, space="DRAM"))

        dma_rot = [nc.sync, nc.scalar, nc.gpsimd]

        # ---- PE warmup: keep the clock ramped while DMAs land ----
        junk_s = wp.tile([KP, 512], f16, name="junk_s")
        nc.vector.memset(junk_s[:], 0.0)
        junk_p = pp.tile([KP, 512], f32, tag="ps", name="junk_p")
        for i in range(24):
            nc.tensor.matmul(junk_p[:], junk_s[:, 0:KP], junk_s[:],
                             start=True, stop=True)

        # ---- input DMAs ----
        # A-plane chunks go on the sync queue only (DMA triggers block the
        # issuing engine); xt + small weights on the gpsimd queue. The
        # scalar/vector queues stay free for casts and mixing.
        # A planes on sync (x2) + gpsimd (x1); the scalar queue carries NO
        # DMAs so the Act engine is free for the fp8 casts from the start.
        apl = [wp.tile([KP, JT, 2, NC], f16, name=f"apl{e}") for e in range(E)]
        nc.sync.dma_start(apl[0][:], ap_in[0])
        nc.gpsimd.dma_start(apl[1][:], ap_in[1])
        nc.sync.dma_start(apl[2][:], ap_in[2])
        xt = xp.tile([KP, JT, 2, 2, 480], f8, name="xt")
        nc.sync.dma_start(xt[:], xt_in[:])
        xto = ep.tile([KP, 4, 2, 480], f16, name="xto")
        nc.gpsimd.dma_start(xto[:], xto_in[:])
        linw = wp.tile([KP, 4, D], f16, name="linw")
        nc.gpsimd.dma_start(linw[:], linw_in[:])
        lbt = wp.tile([D, 1], f32, name="lbt")
        nc.sync.dma_start(lbt[:], lbt_in[:])
        ntt = wp.tile([KP, 1], f32, name="ntt")
        nc.gpsimd.dma_start(ntt[:], ntt_in[:])
        wih = wp.tile([D, 4 * U], f16, name="wih")
        nc.gpsimd.dma_start(wih[:], wih_in[:])
        whh = wp.tile([U, 4 * U], f16, name="whh")
        nc.sync.dma_start(whh[:], whh_in[:])
        gb = wp.tile([1, 4 * U], f32, name="gb")
        nc.gpsimd.dma_start(gb[:], gb_in[:])
        wsc = wp.tile([U, N], f16, name="wsc")
        nc.gpsimd.dma_start(wsc[:], wsc_in[:])
        mp = wp.tile([U, S * BL], mybir.dt.uint8, name="mp")
        nc.sync.dma_start(mp[:], mp_in[:])
        h0t = wp.tile([U, BL], f16, name="h0t")
        nc.gpsimd.dma_start(h0t[:], h0_in[:])
        c0t = wp.tile([U, BL], f32, name="c0t")
        nc.gpsimd.dma_start(c0t[:], c0_in[:])
        wbt = wp.tile([BL, N], f32, name="wbt")
        nc.sync.dma_start(wbt[:], wb_in[:])
        ones240 = wp.tile([1, S * BL], f32, name="ones240")
        nc.vector.memset(ones240[:], 1.0)

        # ---- mixtures: fp16 stt on DVE (fp8 elementwise IO is very slow),
        # then fp16 -> fp8 casts on the otherwise-idle Act engine. cmb
        # m-width padded to 512 so the DoubleRow ldweights dual stride is
        # a multiple of 64.
        t16 = wp.tile([KP, JT, 2, NC], f16, name="t16")
        c16 = wp.tile([KP, JT, 2, NC], f16, name="c16")
        t16p = wp.tile([KP, JT, 2, NC], f16, name="t16p")
        cmb = [wp.tile([KP, JT, 2, 512], f8, name=f"cmb{w}") for w in range(3)]
        # mixes 0 and 1 on DVE (fused stt); mix 2 on Pool (mul/add pairs,
        # no stt support there) — cmb2 is not needed until stage 3, so the
        # slower Pool path has slack and DVE finishes cmb0 ~2x earlier.
        for w, (perm, r0, r1) in enumerate(
                [(p1, r10, r11), (p2, r20, r21)]):
            i0, i1, i2 = perm
            for j0 in (0, 2, 4, 6):
                jsl = slice(j0, j0 + 2)
                nc.vector.scalar_tensor_tensor(
                    t16[:, jsl], apl[i0][:, jsl], r0, apl[i1][:, jsl],
                    AluOpType.mult, AluOpType.add)
                nc.vector.scalar_tensor_tensor(
                    c16[:, jsl], t16[:, jsl], r1, apl[i2][:, jsl],
                    AluOpType.mult, AluOpType.add)
                nc.scalar.copy(cmb[w][:, jsl, :, 0:NC], c16[:, jsl])
        i0, i1, i2 = p3
        # Pool takes j 0-3 only: its queue must drain before the AG1
        # trigger (also on gpsimd) or the collective would be delayed.
        for j0 in (0, 2):
            jsl = slice(j0, j0 + 2)
            nc.gpsimd.tensor_scalar_mul(t16p[:, jsl], apl[i0][:, jsl], r30)
            nc.gpsimd.tensor_tensor(t16p[:, jsl], t16p[:, jsl],
                                    apl[i1][:, jsl], AluOpType.add)
            nc.gpsimd.tensor_scalar_mul(t16p[:, jsl], t16p[:, jsl], r31)
            nc.gpsimd.tensor_tensor(t16p[:, jsl], t16p[:, jsl],
                                    apl[i2][:, jsl], AluOpType.add)
            nc.scalar.copy(cmb[2][:, jsl, :, 0:NC], t16p[:, jsl])
        for j0 in (4, 6):
            jsl = slice(j0, j0 + 2)
            nc.vector.scalar_tensor_tensor(
                t16[:, jsl], apl[i0][:, jsl], r30, apl[i1][:, jsl],
                AluOpType.mult, AluOpType.add)
            nc.vector.scalar_tensor_tensor(
                c16[:, jsl], t16[:, jsl], r31, apl[i2][:, jsl],
                AluOpType.mult, AluOpType.add)
            nc.scalar.copy(cmb[2][:, jsl, :, 0:NC], c16[:, jsl])

        # ---- chain stages ----
        def stage(cmb_t, prep, rhs_of, s_idx):
            ps = [pp.tile([KP, 2, 512], f32, tag="ps", name=f"s{s_idx}p{m}")
                  for m in range(4)]
            for j in range(JT):
                prep(j)
                for m in range(4):
                    lhs = cmb_t[:, j, :, m * KP:(m + 1) * KP]
                    for hb in range(2):
                        nc.tensor.matmul(
                            ps[m][:, hb, 0:480], lhs, rhs_of(j, hb),
                            start=(j == 0), stop=(j == JT - 1), perf_mode=DR)
            return ps

        def evac_ag(ps, scale, nm):
            yo = ep.tile([KP, 2, 2, 2, 480], f8, tag="yo", name=f"yo{nm}")
            for m in range(4):
                if m % 2 == 0:
                    nc.vector.tensor_scalar_mul(yo[:, m // 2, m % 2],
                                                ps[m][:, :, 0:480], scale)
                else:
                    nc.scalar.mul(yo[:, m // 2, m % 2], ps[m][:, :, 0:480],
                                  scale)
            agi = dr.tile([KP, 2, 2, 2, 480], f8, name=f"agi{nm}")
            ago = dr.tile([NG, KP, 2, 2, 2, 480], f8, name=f"ago{nm}")
            nc.sync.dma_start(agi[:], yo[:])
            nc.gpsimd.collective_compute(
                "AllGather", mybir.AluOpType.bypass,
                replica_groups=G1,
                ins=[agi[:].opt()], outs=[ago[:].opt()])
            return ago

        rt = [xp.tile([KP, 2, 2, 480], f8, name=f"rt{j}") for j in range(JT)]

        def mk_loader(ago):
            def prep(j):
                dma_rot[j % 3].dma_start(rt[j][:], ago[j // 2, :, j % 2])
            return prep

        ps = stage(cmb[0], lambda j: None,
                   lambda j, hb: xt[:, j, :, hb, :], 1)
        ago1 = evac_ag(ps, s1, "1")
        ps = stage(cmb[1], mk_loader(ago1),
                   lambda j, hb: rt[j][:, :, hb, :], 2)
        ago2 = evac_ag(ps, s2, "2")
        ps3 = stage(cmb[2], mk_loader(ago2),
                    lambda j, hb: rt[j][:, :, hb, :], 3)

        # ---- enc = x*scale/S_ENC + relu(y3 - thr)/S_ENC (fp16) ----
        enc = ep.tile([KP, 4, 2, 480], f16, name="enc")
        for m in range(4):
            nc.scalar.activation(enc[:, m], ps3[m][:, :, 0:480], AF.Relu,
                                 bias=ntt[:], scale=s3)
        for m in range(4):
            eng = nc.vector if m % 2 == 0 else nc.gpsimd
            eng.tensor_tensor(enc[:, m], enc[:, m], xto[:, m], AluOpType.add)
        if dbg:
            nc.sync.dma_start(d_enc[:], enc[:])

        # ---- basket partial: lin_w_shard^T @ enc -> [128, 960] psum ----
        psL = pp.tile([D, 2, 512], f32, tag="ps", name="psL")
        for kt in range(4):
            for hb in range(2):
                nc.tensor.matmul(psL[:, hb, 0:480], linw[:, kt], enc[:, kt, hb],
                                 start=(kt == 0), stop=(kt == 3))
        bkp = ep.tile([D, 2, 480], f32, name="bkp")
        nc.vector.tensor_copy(bkp[:], psL[:, :, 0:480])

        rs_i = dr.tile([NG, D, S * BL], f32, name="rsi")
        rs_o = dr.tile([D, S * BL], f32, name="rso")
        for r in range(NG):
            dma_rot[r % 3].dma_start(
                rs_i[r], bkp[:, r // 2, (r % 2) * 240:(r % 2) * 240 + 240])
        nc.gpsimd.collective_compute(
            "ReduceScatter", mybir.AluOpType.add,
            replica_groups=G1,
            ins=[rs_i[:].opt()], outs=[rs_o[:].opt()])

        bsum = wp.tile([D, S * BL], f32, name="bsum")
        nc.sync.dma_start(bsum[:], rs_o[:])
        bkc = wp.tile([D, S * BL], f16, name="bkc")
        nc.scalar.activation(bkc[:], bsum[:], AF.Relu, bias=lbt[:],
                             scale=1.0 / S_ENC)
        if dbg:
            nc.sync.dma_start(d_bkc[:], bkc[:])

        # ---- g1 = Wih^T basket + bias, evacuated to SBUF ----
        psD = pp.tile([U, 4, 256], f32, tag="ps", name="psD")
        for mg in range(4):
            nc.tensor.matmul(psD[:, mg, 0:S * BL], wih[:, mg * U:(mg + 1) * U],
                             bkc[:], start=True, stop=False)
            nc.tensor.matmul(psD[:, mg, 0:S * BL], gb[:, mg * U:(mg + 1) * U],
                             ones240[:], start=False, stop=True)
        g1t = wp.tile([U, 4, S * BL], f32, name="g1t")
        nc.vector.tensor_copy(g1t[:], psD[:, :, 0:S * BL])

        # ---- LSTM: 2 groups x 4 batches, [U, batch] layout.
        # tanh done as sigmoid: t2 = 4*sig(2g~)-2 tracks 2*tanh (weights for
        # g~ pre-doubled on host), c' = 2c, h = 2*sig_o*sig(c') - sig_o.
        hT = h0t
        cT = c0t
        la = wp.tile([U, BL], f16, name="la")
        nc.vector.memset(la[:], 0.0)
        gall = [wp.tile([U, 4, GW], f32, name=f"gall{g}") for g in range(2)]
        sall = [wp.tile([U, 4, GW], f32, name=f"sall{g}") for g in range(2)]
        t2g = [wp.tile([U, GW], f32, name=f"t2g{g}") for g in range(2)]
        tgg = [wp.tile([U, GW], f32, name=f"tgg{g}") for g in range(2)]
        scg = [wp.tile([U, GW], f32, name=f"scg{g}") for g in range(2)]
        ug = [wp.tile([U, GW], f32, name=f"ug{g}") for g in range(2)]

        for t in range(S):
            for g in range(2):
                cs = t * BL + g * GW
                bs = slice(g * GW, (g + 1) * GW)
                psE = pp.tile([U, 4, GW], f32, tag="ps", name=f"psE{t}_{g}")
                for mg in range(4):
                    nc.tensor.matmul(psE[:, mg], whh[:, mg * U:(mg + 1) * U],
                                     hT[:, bs], start=True, stop=True)
                nc.vector.scalar_tensor_tensor(
                    gall[g][:], psE[:], 1.0, g1t[:, :, cs:cs + GW],
                    AluOpType.mult, AluOpType.add)
                nc.scalar.activation(sall[g][:], gall[g][:], AF.Sigmoid)
                nc.vector.tensor_scalar(
                    out=t2g[g][:], in0=sall[g][:, 3], scalar1=4.0, scalar2=2.0,
                    op0=mybir.AluOpType.mult, op1=mybir.AluOpType.subtract)
                nc.vector.tensor_tensor(tgg[g][:], sall[g][:, 0], t2g[g][:],
                                        AluOpType.mult)
                nc.gpsimd.tensor_tensor(cT[:, bs], sall[g][:, 1], cT[:, bs],
                                        AluOpType.mult)
                nc.vector.tensor_tensor(cT[:, bs], cT[:, bs], tgg[g][:],
                                        AluOpType.add)
                nc.scalar.activation(scg[g][:], cT[:, bs], AF.Sigmoid)
                nc.vector.tensor_tensor(ug[g][:], sall[g][:, 2], scg[g][:],
                                        AluOpType.mult)
                nc.vector.scalar_tensor_tensor(
                    hT[:, bs], ug[g][:], 2.0, sall[g][:, 2],
                    AluOpType.mult, AluOpType.subtract)
            nc.vector.copy_predicated(la[:], mp[:, t * BL:(t + 1) * BL], hT[:])
        if dbg:
            nc.sync.dma_start(d_last[:], la[:])

        # ---- scoring ----
        psF = [pp.tile([BL, 2, 512], f32, tag="ps", name=f"psF{i}")
               for i in range(2)]
        for b in range(4):
            nc.tensor.matmul(psF[b // 2][:, b % 2, 0:500], la[:],
                             wsc[:, b * 500:(b + 1) * 500],
                             start=True, stop=True)
        probs = wp.tile([BL, N], f32, name="probs")
        for b in range(4):
            nc.scalar.activation(probs[:, b * 500:(b + 1) * 500],
                                 psF[b // 2][:, b % 2, 0:500], AF.Sigmoid)
        nc.vector.tensor_tensor(probs[:, 0:1000], probs[:, 0:1000],
                                wbt[:, 0:1000], AluOpType.mult)
        nc.gpsimd.tensor_tensor(probs[:, 1000:2000], probs[:, 1000:2000],
                                wbt[:, 1000:2000], AluOpType.mult)
        nc.sync.dma_start(out_t[:], probs[:])

    nc.compile()
    return nc


def kernel(A, seq_len, seqs, h0, c0, W1a, W1b, W2, lin_w, lin_b,
           Wih, Whh, bih, bhh, Wscore, I_B, threshold):
    import concourse.mybir as mybir

    f32, f16 = np.float32, np.float16
    f8 = mybir.dt.np(mybir.dt.float8e4)
    A = np.asarray(A, f32)
    seqs = np.asarray(seqs, f32)
    seq_len = np.asarray(seq_len).astype(np.int64)
    lin_w = np.asarray(lin_w, f32)
    lin_b = np.asarray(lin_b, f32)
    Wih = np.asarray(Wih, f32)
    Whh = np.asarray(Whh, f32)
    bias = np.asarray(bih, f32) + np.asarray(bhh, f32)
    Wscore = np.asarray(Wscore, f32)
    scale = np.maximum(np.asarray(I_B, f32), 0.0)
    thr = float(np.asarray(threshold, f32).reshape(-1)[0])

    sa = _softmax_row0(W1a)
    sb = _softmax_row0(W1b)
    s2 = _softmax_row0(W2)
    mixes = (sa, sb, s2)

    dbg = bool(os.environ.get("GTN_DBG"))
    key = (sa.tobytes(), sb.tobytes(), s2.tobytes(), dbg)
    if key not in _CACHE:
        _CACHE.clear()
        _CACHE[key] = _build(mixes, dbg=dbg)
    nc = _CACHE[key]

    # column permutation: col(b, s) = (b//32)*960 + ((b%32)//8)*240 + s*8 + b%8
    b_idx = np.arange(B)[:, None]
    s_idx = np.arange(S)[None, :]
    cols_of = ((b_idx // BH) * BSH + ((b_idx % BH) // BL) * (S * BL)
               + s_idx * BL + (b_idx % BL)).reshape(-1)
    x = seqs.reshape(B * S, N)
    xTp = np.empty((N, B * S), f32)
    xTp[:, cols_of] = x.T

    # gate reorder (i, f, o, g~), g~ weights x2 for tanh-as-sigmoid
    gidx = np.r_[0:2 * U, 3 * U:4 * U, 2 * U:3 * U]
    wih_h = Wih[gidx] * S_ENC
    wih_h[3 * U:] *= 2.0
    whh_h = Whh[gidx].copy()
    whh_h[3 * U:] *= 2.0
    gb_h = bias[gidx].copy()
    gb_h[3 * U:] *= 2.0

    WihT = np.ascontiguousarray(wih_h.T.astype(f16))
    WhhT = np.ascontiguousarray(whh_h.T.astype(f16))
    gbr = np.ascontiguousarray(gb_h[None, :].astype(f32))
    WscoreT = np.ascontiguousarray(Wscore.T.astype(f16))
    wb_row = ((1.0 - ALPHA) + ALPHA * scale).astype(f32)
    wb = np.ascontiguousarray(np.broadcast_to(wb_row[None, :], (BL, N)))
    lbt = np.ascontiguousarray((lin_b / S_ENC).reshape(D, 1))
    ntt = np.full((KP, 1), -thr / S_ENC, f32)
    h0T = np.asarray(h0, f32)[0].T
    c0T = np.asarray(c0, f32)[0].T * 2.0     # c' = 2c

    in_maps = []
    for c in range(P):
        g, h = c % NG, c // NG
        gsl = slice(g * NC, (g + 1) * NC)
        hsl = slice(h * BSH, (h + 1) * BSH)
        # A column shard, DoubleRow-packed [KP, JT, 2, NC]
        apc = np.ascontiguousarray(
            A[:, gsl, :].transpose(2, 0, 1)          # [E, 2000, 500]
            .reshape(E, JT, 2, KP, NC).transpose(0, 3, 1, 2, 4)
            .reshape(E, KP, JT * 2 * NC).astype(f16))
        xt8 = np.ascontiguousarray(
            xTp[:, hsl].reshape(JT, 2, KP, BSH).transpose(2, 0, 1, 3)
            .reshape(KP, JT, 2, 2, 480).astype(f8))
        xto = np.ascontiguousarray(
            (xTp[gsl, hsl.start:hsl.stop] * (scale[gsl, None] / S_ENC))
            .reshape(4, KP, BSH).transpose(1, 0, 2)
            .reshape(KP, 4, 2, 480).astype(f16))
        linw = np.ascontiguousarray(
            (lin_w[:, gsl].T * S_ENC).reshape(4, KP, D).astype(f16)
            .transpose(1, 0, 2))
        mpm = np.zeros((S * BL,), np.uint8)
        for bl in range(BL):
            gb_ = h * BH + g * BL + bl
            t_sel = int(seq_len[gb_]) - 1
            mpm[t_sel * BL + bl] = 1
        mpP = np.ascontiguousarray(np.broadcast_to(mpm[None, :], (U, S * BL)))
        bsl = slice(h * BH + g * BL, h * BH + g * BL + BL)
        in_maps.append({
            "ap": apc, "xt": xt8, "xto": xto, "linw": linw,
            "lbt": lbt, "ntt": ntt, "wih": WihT, "whh": WhhT, "gb": gbr,
            "wsc": WscoreT, "mp": mpP, "wb": wb,
            "h0t": np.ascontiguousarray(h0T[:, bsl].astype(f16)),
            "c0t": np.ascontiguousarray(c0T[:, bsl]),
        })

    from concourse.bass_utils import run_bass_kernel_spmd
    trace = bool(os.environ.get("GTN_TRACE"))
    res = run_bass_kernel_spmd(nc, in_maps, core_ids=list(range(P)), trace=trace)
    if trace and res.exec_time_ns is not None:
        kernel.last_exec_time_ns = res.exec_time_ns
        kernel.last_trace = res.instructions_and_trace
    if dbg:
        kernel.last_results = res.results
    predict = np.concatenate([res.results[c]["out"] for c in range(P)], axis=0)
    return predict.astype(f32)


kernel.last_exec_time_ns = None
kernel.last_trace = None
